# revision 1
# baseline (speedup 1.0000x reference)
"""GatedDeltaNet attention kernel for 8 Trainium2 NeuronCores.

Problem: B=2, L=2048, D=1024, H=16 heads (Dh=64).
  q,k,v = x@Wq, x@Wk, x@Wv ; beta = sigmoid(x@Wb + bb)
  q,k l2-normalized per head; out[l] = sum_{t<=l} beta_t <qh_l,kh_t> vh_t
  y = out @ Wo

Sharding: 8 cores = 2 batches x 4 head-groups (4 heads each). Each core
computes its batch/heads slice end-to-end including a partial y (contraction
over its 256 Wo rows); host sums the 4 partials per batch.

Device algorithm (per core), all matmuls in float32r (full PE rate):
  P1: qT/kT = W^T-style projections into [d', l] layout (lhsT=W, rhs=xT),
      v into [t, e] layout (lhsT=x-block, rhs=Wvb with beta logits fused as
      4 extra columns). l2-norm factors via Square + selector-matmul
      partition reductions; 1/|k_t| and beta fold into v ("vtilde"),
      1/|q_l| folds into the attention-output copy, with its per-head
      row-broadcast materialized by K=1 indicator matmuls on TensorE.
  P2: per head-pair, causal-skipped score tiles ST[t,l] (2-head packed via
      row tile_position), PSUM->SBUF copy is plain (off-diag) or one
      mask-multiply (diagonal); out2 accumulated over t-blocks with 2-head
      column packing.
  P3: yT = Wo^T @ attnT, copied to SBUF and DMA'd out.
"""

import numpy as np

P = 128
L = 2048
D = 1024
H = 16
KS = D // P        # 8 contraction subtiles
NT = L // P        # 16 t-blocks
CH = 512
NCH = L // CH      # 4 l-chunks
DH = 64
HC = 4             # heads per core
NP = HC // 2       # head pairs per core
NCORES = 8
GROUPS = NCORES // 2  # head groups (4)

_CACHE = {}

# out2 (score @ v) matmuls in bf16: enables 2-head column packing on the
# PE array (fp32r matmuls cannot write PSUM at partition 64).
OUT2_BF16 = True
DEBUG_DUMP = False


def _build_nc():
    import concourse.bass as bass  # noqa: F401
    import concourse.tile as tile
    import concourse.mybir as mybir
    from concourse import bacc
    from contextlib import ExitStack

    F32 = mybir.dt.float32
    F32R = mybir.dt.float32r
    AF = mybir.ActivationFunctionType
    OP = mybir.AluOpType

    nc = bacc.Bacc(
        "TRN2", target_bir_lowering=False, debug=False, num_devices=NCORES
    )

    xT = nc.dram_tensor("xT", [KS, P, L], F32R, kind="ExternalInput")
    wq = nc.dram_tensor("wq", [KS, P, NP * P], F32R, kind="ExternalInput")
    wk = nc.dram_tensor("wk", [KS, P, NP * P], F32R, kind="ExternalInput")
    wvb = nc.dram_tensor("wvb", [KS, P, HC * DH + HC], F32R, kind="ExternalInput")
    wo = nc.dram_tensor("wo", [NP, P, D], F32R, kind="ExternalInput")
    sel = nc.dram_tensor("sel", [P, 2], F32R, kind="ExternalInput")
    bbb = nc.dram_tensor("bbb", [P, HC], F32, kind="ExternalInput")
    masks = nc.dram_tensor("masks", [P, P], F32, kind="ExternalInput")
    ind = nc.dram_tensor("ind", [2, P], F32R, kind="ExternalInput")
    yT = nc.dram_tensor("yT", [D, L], F32, kind="ExternalOutput")
    dbg = {}
    if DEBUG_DUMP:
        BF16_ = mybir.dt.bfloat16
        DT2_ = BF16_ if OUT2_BF16 else F32R
        for nm, shp, dt_ in [("d_qT0", [P, L], F32R), ("d_kT0", [P, L], F32R),
                        ("d_vt", [P, NT * HC * DH], DT2_), ("d_factor", [P, NT * HC], F32),
                        ("d_rnq0", [1, L], F32), ("d_rnqb0", [P, L], F32),
                        ("d_attnT0", [P, L], F32R), ("d_rnk", [P, NT * 2 * NP], F32)]:
            dbg[nm] = nc.dram_tensor(nm, shp, dt_, kind="ExternalOutput")

    NV = HC * DH  # 256

    with tile.TileContext(nc) as tc:
        with ExitStack() as ctx:
            pconst = ctx.enter_context(tc.tile_pool(name="const", bufs=1))
            pmain = ctx.enter_context(tc.tile_pool(name="main", bufs=1))

            # running modeled-busy accumulators for DVE vs ACT copy balancing
            eng_load = {"dve": 0.0, "act": 0.0}

            def note(eng, t):
                eng_load[eng] += t

            def bal_copy(out_ap, in_ap, fd):
                cd = (120 + fd) / 0.96
                ca = (210 + fd) / 1.05
                if eng_load["dve"] + cd <= eng_load["act"] + ca:
                    note("dve", cd)
                    nc.vector.tensor_copy(out_ap, in_ap)
                else:
                    note("act", ca)
                    nc.scalar.activation(out_ap, in_ap, AF.Copy)

            sel_sb = pconst.tile([P, 2], F32R, tag="sel", name="sel")
            nc.sync.dma_start(sel_sb[:], sel.ap())
            bbb_sb = pconst.tile([P, HC], F32, tag="bbb", name="bbb")
            nc.sync.dma_start(bbb_sb[:], bbb.ap())
            mask_sb = pconst.tile([P, P], F32, tag="mask", name="mask")
            wo_sb = pconst.tile([P, NP, D], F32R, tag="wo", name="wo")
            ind_sb = pconst.tile([2, P], F32R, tag="ind", name="ind")
            # (their DMAs are issued after the projection inputs below —
            # they are not needed until P2/P3)

            qT = [pmain.tile([P, L], F32R, tag=f"qT{p}", name=f"qT{p}") for p in range(NP)]
            kT = [pmain.tile([P, L], F32R, tag=f"kT{p}", name=f"kT{p}") for p in range(NP)]
            BF16 = mybir.dt.bfloat16
            DT2 = BF16 if OUT2_BF16 else F32R
            vt = pmain.tile([P, NT, NV], DT2, tag="vt", name="vt")
            factor = pmain.tile([P, NT, HC], F32, tag="factor", name="factor")
            rnk_sb = pmain.tile([P, NT, 2 * NP], F32, tag="rnk", name="rnk")
            # one [2, L] tile per head-pair (head rows at partitions 0/1)
            rnq_sb = [pmain.tile([2, L], F32R, tag=f"rnq{p}", name=f"rnq{p}") for p in range(NP)]

            # ---------------- P1: projections ----------------
            with ExitStack() as p1:
                px = p1.enter_context(tc.tile_pool(name="x", bufs=KS))
                pw = p1.enter_context(tc.tile_pool(name="w", bufs=1))
                psq = p1.enter_context(tc.tile_pool(name="sq", bufs=3))
                ptmp = p1.enter_context(tc.tile_pool(name="tmp", bufs=4))
                ppA = p1.enter_context(
                    tc.tile_pool(name="ppA", bufs=5, space="PSUM")
                )
                ppB = p1.enter_context(
                    tc.tile_pool(name="ppB", bufs=2, space="PSUM")
                )
                ppC = p1.enter_context(
                    tc.tile_pool(name="ppC", bufs=1, space="PSUM")
                )
                # all 32 k-norm selector matmuls land in one PSUM bank as
                # [128, tb, pair*2+h] columns; overwrite-on-pending-zero
                # makes disjoint-column writes of one started group safe
                ssk_all = ppC.tile([P, NT, 2 * NP], F32, tag="sskall", name="sskall")
                ssk_n = [0]

                # Fine-grained input DMAs so matmul deps release early
                # (byte-range dep tracking): per-ksub weight slices and
                # per-chunk x slices, interleaved in consumption order and
                # spread across the SP/ACT/GPSIMD DMA queues.
                wq_sb = pw.tile([P, KS, NP * P], F32R, tag="wq", name="wq")
                wk_sb = pw.tile([P, KS, NP * P], F32R, tag="wk", name="wk")
                wvb_sb = pw.tile([P, KS, NV + HC], F32R, tag="wvb", name="wvb")
                x_sb = [px.tile([P, L], F32R, tag="x", name="x")
                        for _ in range(KS)]
                x_eng = [nc.sync, nc.scalar]
                for ks in range(KS):
                    nc.gpsimd.dma_start(wq_sb[:, ks, :], wq.ap()[ks])
                    for c in range(NCH):
                        x_eng[(ks * NCH + c) % 2].dma_start(
                            x_sb[ks][:, c * CH:(c + 1) * CH],
                            xT.ap()[ks][:, c * CH:(c + 1) * CH],
                        )
                for ks in range(KS):
                    nc.gpsimd.dma_start(wk_sb[:, ks, :], wk.ap()[ks])
                    nc.gpsimd.dma_start(wvb_sb[:, ks, :], wvb.ap()[ks])

                # deferred P2/P3 constants
                nc.gpsimd.dma_start(ind_sb[:], ind.ap())
                nc.gpsimd.dma_start(mask_sb[:], masks.ap())
                nc.gpsimd.dma_start(wo_sb[:], wo.ap().rearrange("s p d -> p s d"))

                # q and k projections, with norm-factor chains
                for w_sb, dst, is_q in ((wq_sb, qT, True), (wk_sb, kT, False)):
                    for pair in range(NP):
                        ps = [ppA.tile([P, CH], F32, tag="mm", name="mm") for _ in range(NCH)]
                        for ks in range(KS):
                            lhsT = w_sb[:, ks, pair * P:(pair + 1) * P]
                            for c in range(NCH):
                                nc.tensor.matmul(
                                    ps[c][:],
                                    lhsT,
                                    x_sb[ks][:, c * CH:(c + 1) * CH],
                                    start=(ks == 0),
                                    stop=(ks == KS - 1),
                                )
                        for c in range(NCH):
                            bal_copy(
                                dst[pair][:, c * CH:(c + 1) * CH], ps[c][:], CH
                            )
                            sq = psq.tile([P, CH], F32R, tag="sq", name="sq")
                            nc.scalar.activation(sq[:], ps[c][:], AF.Square)
                            note("act", (172 + CH) / 1.2)
                            if is_q:
                                ss = ppB.tile([2, CH], F32, tag="ss", name="ss")
                                nc.tensor.matmul(
                                    ss[:], sel_sb[:], sq[:],
                                    start=True, stop=True,
                                )
                                nc.scalar.activation(
                                    rnq_sb[pair][:, c * CH:(c + 1) * CH],
                                    ss[:], AF.Abs_reciprocal_sqrt,
                                )
                                note("act", (172 + CH) / 1.2)
                            else:
                                for tr in range(CH // P):
                                    tb = c * (CH // P) + tr
                                    nc.tensor.matmul(
                                        ssk_all[:, tb, pair * 2:pair * 2 + 2],
                                        sq[:, tr * P:(tr + 1) * P],
                                        sel_sb[:],
                                        start=(ssk_n[0] == 0),
                                        stop=(ssk_n[0] == 2 * NP * NT - 1),
                                        skip_group_check=True,
                                    )
                                    ssk_n[0] += 1

                # k-norm: single reciprocal + sqrt over the packed bank
                nc.scalar.activation(
                    rnk_sb[:].rearrange("p a b -> p (a b)"),
                    ssk_all[:].rearrange("p a b -> p (a b)"),
                    AF.Abs_reciprocal_sqrt)
                note("act", 300.0)

                # v projection (+ fused beta logits) -> vtilde, plus a
                # second GEMM off the same stationary x-block producing k in
                # normal [t, d] layout (bf16) for the inter-chunk state path
                kn = pmain.tile([P, NT, NV], BF16, tag="kn", name="kn")
                for tb in range(NT):
                    psv = ppA.tile([P, NV + HC], F32, tag="mm", name="mm")
                    need_kn = tb < NT - NT // NCH  # chunk 3 never enters the state
                    if need_kn:
                        psk = ppA.tile([P, NV], F32, tag="mm", name="mmk")
                    for ks in range(KS):
                        nc.tensor.matmul(
                            psv[:],
                            x_sb[ks][:, tb * P:(tb + 1) * P],
                            wvb_sb[:, ks, :],
                            start=(ks == 0),
                            stop=(ks == KS - 1),
                        )
                        if need_kn:
                            nc.tensor.matmul(
                                psk[:],
                                x_sb[ks][:, tb * P:(tb + 1) * P],
                                wk_sb[:, ks, :],
                                start=(ks == 0),
                                stop=(ks == KS - 1),
                            )
                    if need_kn:
                        bal_copy(kn[:, tb, :], psk[:], NV)
                    bl = ptmp.tile([P, HC], F32, tag="bl", name="bl")
                    nc.vector.tensor_tensor(
                        bl[:], psv[:, NV:], bbb_sb[:], OP.add
                    )
                    bs = ptmp.tile([P, HC], F32, tag="bs", name="bs")
                    nc.scalar.activation(bs[:], bl[:], AF.Sigmoid)
                    note("act", 180.0)
                    note("dve", 300.0)
                    nc.vector.tensor_tensor(
                        factor[:, tb, :], bs[:], rnk_sb[:, tb, :], OP.mult
                    )
                    nc.vector.tensor_tensor(
                        vt[:, tb, :].rearrange("p (h e) -> p h e", e=DH),
                        psv[:, :NV].rearrange("p (h e) -> p h e", e=DH),
                        factor[:, tb, :, None].to_broadcast((P, HC, DH)),
                        OP.mult,
                    )
                    note("dve", (120 + NV) / 0.96)

            # ---------------- P2 + P3 ----------------
            with ExitStack() as p2:
                p2m = p2.enter_context(tc.tile_pool(name="p2m", bufs=1))
                pst = p2.enter_context(tc.tile_pool(name="stbuf", bufs=8))
                pyout = p2.enter_context(tc.tile_pool(name="yout", bufs=6))
                ppst = p2.enter_context(
                    tc.tile_pool(name="ppst", bufs=4, space="PSUM")
                )
                ppo2 = p2.enter_context(
                    tc.tile_pool(name="ppo2", bufs=2, space="PSUM")
                )
                pps_s = p2.enter_context(
                    tc.tile_pool(name="pps_s", bufs=1, space="PSUM")
                )

                rnqb = [p2m.tile([P, L], F32, tag=f"rnqb{p}", name=f"rnqb{p}") for p in range(NP)]
                attnT = [p2m.tile([P, L], F32R, tag=f"attnT{p}", name=f"attnT{p}") for p in range(NP)]
                # broadcast rnq rows across partitions via two K=1
                # accumulating matmuls against host indicator rows
                for pair in range(NP):
                    for c in range(NCH):
                        bc = ppst.tile([P, CH], F32, tag="st", name="bc")
                        nc.tensor.matmul(
                            bc[:],
                            ind_sb[:],
                            rnq_sb[pair][:, c * CH:(c + 1) * CH],
                            start=True, stop=True,
                        )
                        bal_copy(rnqb[pair][:, c * CH:(c + 1) * CH], bc[:], CH)

                # running DeltaNet state S[d, e] per pair (accumulated in
                # PSUM across chunk boundaries) + bf16 copies of S and qT
                # for the inter-chunk matmuls
                s_ps = [pps_s.tile([P, NV], F32, tag=f"sps{p}", name=f"sps{p}")
                        for p in range(NP)]
                s_sb = [p2m.tile([P, NV], BF16, tag=f"ssb{p}", name=f"ssb{p}")
                        for p in range(NP)]
                qTb = [p2m.tile([P, L], BF16, tag=f"qTb{p}", name=f"qTb{p}")
                       for p in range(NP)]
                for pair in range(NP):
                    for c in range(1, NCH):
                        bal_copy(qTb[pair][:, c * CH:(c + 1) * CH],
                                 qT[pair][:, c * CH:(c + 1) * CH], CH)

                for c in range(NCH):
                    if c > 0:
                        # fold chunk c-1 into the state, snapshot to bf16
                        for pair in range(NP):
                            for tsub in range(4):
                                tb = (c - 1) * 4 + tsub
                                nc.tensor.matmul(
                                    s_ps[pair][:],
                                    kn[:, tb, pair * P:(pair + 1) * P],
                                    vt[:, tb, :],
                                    start=(c == 1 and tsub == 0),
                                    stop=(c == NCH - 1 and tsub == 3),
                                    skip_group_check=True,
                                )
                            bal_copy(s_sb[pair][:], s_ps[pair][:], NV)
                    for pair in range(NP):
                        o2 = ppo2.tile([P, CH], F32, tag="o2", name="o2")
                        if c > 0:
                            # inter-chunk contribution: o2 = S_h^T-applied q
                            for hh in range(2):
                                h = 2 * pair + hh
                                nc.tensor.matmul(
                                    o2[64 * hh:64 * (hh + 1), :],
                                    s_sb[pair][
                                        64 * hh:64 * (hh + 1),
                                        h * DH:(h + 1) * DH,
                                    ],
                                    qTb[pair][
                                        64 * hh:64 * (hh + 1),
                                        c * CH:(c + 1) * CH,
                                    ],
                                    start=True, stop=False,
                                    tile_position=(64 * hh, 64 * hh),
                                    skip_group_check=True,
                                )
                        for T in range(4 * c, 4 * c + 4):
                            j = T - 4 * c
                            lo = P * j if j > 0 else 0
                            stps = [
                                ppst.tile([P, CH], F32, tag="st", name="st")
                                for _ in range(2)
                            ]
                            for hh in range(2):
                                nc.tensor.matmul(
                                    stps[hh][:, lo:CH],
                                    kT[pair][
                                        64 * hh:64 * (hh + 1), T * P:(T + 1) * P
                                    ],
                                    qT[pair][
                                        64 * hh:64 * (hh + 1),
                                        c * CH + lo:(c + 1) * CH,
                                    ],
                                    start=True, stop=True,
                                )
                            st_sb = [
                                pst.tile([P, CH], DT2, tag="st_sb", name="st_sb")
                                for _ in range(2)
                            ]
                            for hh in range(2):
                                    # triangular 128-col block at the causal
                                    # frontier; rest is plain copy
                                    nc.vector.tensor_tensor(
                                        st_sb[hh][:, lo:lo + P],
                                        stps[hh][:, lo:lo + P],
                                        mask_sb[:], OP.mult,
                                    )
                                    note("dve", (120 + P) / 0.96)
                                    if lo + P < CH:
                                        bal_copy(
                                            st_sb[hh][:, lo + P:CH],
                                            stps[hh][:, lo + P:CH],
                                            CH - lo - P,
                                        )
                            for hh in range(2):
                                h = 2 * pair + hh
                                nc.tensor.matmul(
                                    o2[64 * hh:64 * (hh + 1), lo:CH],
                                    vt[:, T, h * DH:(h + 1) * DH],
                                    st_sb[hh][:, lo:CH],
                                    start=(c == 0 and T == 0),
                                    stop=(T == 4 * c + 3),
                                    tile_position=(0, 64 * hh),
                                    skip_group_check=True,
                                )
                        nc.vector.tensor_tensor(
                            attnT[pair][:, c * CH:(c + 1) * CH],
                            o2[:],
                            rnqb[pair][:, c * CH:(c + 1) * CH],
                            OP.mult,
                        )
                        note("dve", (120 + CH) / 0.96)

                    # P3 for this chunk: yT[:, c] = wo^T @ attnT[:, c]
                    for m in range(D // P):
                        py = ppst.tile([P, CH], F32, tag="st", name="y")
                        for pair in range(NP):
                            nc.tensor.matmul(
                                py[:],
                                wo_sb[:, pair, m * P:(m + 1) * P],
                                attnT[pair][:, c * CH:(c + 1) * CH],
                                start=(pair == 0),
                                stop=(pair == NP - 1),
                            )
                        yo = pyout.tile([P, CH], F32, tag="yo", name="yo")
                        bal_copy(yo[:], py[:], CH)
                        nc.sync.dma_start(
                            yT.ap()[m * P:(m + 1) * P, c * CH:(c + 1) * CH],
                            yo[:],
                        )

                if DEBUG_DUMP:
                    nc.sync.dma_start(dbg["d_qT0"].ap(), qT[0][:])
                    nc.sync.dma_start(dbg["d_kT0"].ap(), kT[0][:])
                    nc.sync.dma_start(dbg["d_vt"].ap(), vt[:].rearrange("p a b -> p (a b)"))
                    nc.sync.dma_start(dbg["d_factor"].ap(), factor[:].rearrange("p a b -> p (a b)"))
                    nc.sync.dma_start(dbg["d_rnq0"].ap(), rnq_sb[0][:])
                    nc.sync.dma_start(dbg["d_rnqb0"].ap(), rnqb[0][:])
                    nc.sync.dma_start(dbg["d_attnT0"].ap(), attnT[0][:])
                    nc.sync.dma_start(dbg["d_rnk"].ap(), rnk_sb[:].rearrange("p a b -> p (a b)"))

    nc.compile()
    return nc


def get_nc():
    if "nc" not in _CACHE:
        _CACHE["nc"] = _build_nc()
    return _CACHE["nc"]


def make_core_inputs(x, Wq, Wk, Wv, Wo, Wb, bb):
    """Build the 8 per-core input maps from full inputs."""
    x = np.asarray(x, dtype=np.float32)
    Wq = np.asarray(Wq, dtype=np.float32)
    Wk = np.asarray(Wk, dtype=np.float32)
    Wv = np.asarray(Wv, dtype=np.float32)
    Wo = np.asarray(Wo, dtype=np.float32)
    Wb = np.asarray(Wb, dtype=np.float32)
    bb = np.asarray(bb, dtype=np.float32)

    selm = np.zeros((P, 2), dtype=np.float32)
    selm[:64, 0] = 1.0
    selm[64:, 1] = 1.0
    indm = np.zeros((2, P), dtype=np.float32)
    indm[0, :64] = 1.0
    indm[1, 64:] = 1.0
    masks = (np.arange(P)[:, None] <= np.arange(P)[None, :]).astype(np.float32)

    in_maps = []
    for core in range(NCORES):
        b, g = divmod(core, GROUPS)
        hs = slice(NV_G * g, NV_G * (g + 1))
        bs = slice(HC * g, HC * (g + 1))
        xTc = np.ascontiguousarray(x[b].T).reshape(KS, P, L)
        wqc = np.ascontiguousarray(Wq[:, hs]).reshape(KS, P, NP * P)
        wkc = np.ascontiguousarray(Wk[:, hs]).reshape(KS, P, NP * P)
        wvbc = np.ascontiguousarray(
            np.concatenate([Wv[:, hs], Wb[:, bs]], axis=1)
        ).reshape(KS, P, NV_G + HC)
        woc = np.ascontiguousarray(Wo[hs, :]).reshape(NP, P, D)
        bbbc = np.ascontiguousarray(np.tile(bb[bs][None, :], (P, 1)))
        in_maps.append(
            {
                "xT": xTc,
                "wq": wqc,
                "wk": wkc,
                "wvb": wvbc,
                "wo": woc,
                "sel": selm,
                "bbb": bbbc,
                "masks": masks,
                "ind": indm,
            }
        )
    return in_maps


NV_G = HC * DH  # 256 columns per head group


def kernel(x, Wq, Wk, Wv, Wo, Wb, bb):
    from concourse.bass_utils import run_bass_kernel_spmd

    nc = get_nc()
    in_maps = make_core_inputs(x, Wq, Wk, Wv, Wo, Wb, bb)
    try:
        res = run_bass_kernel_spmd(nc, in_maps, core_ids=list(range(NCORES)))
    except Exception:
        # transient NRT wedges (e.g. NRT_EXEC_UNIT_UNRECOVERABLE) clear on
        # a fresh attempt; retry once before giving up
        res = run_bass_kernel_spmd(nc, in_maps, core_ids=list(range(NCORES)))
    B = 2
    y = np.zeros((B, L, D), dtype=np.float32)
    for core in range(NCORES):
        b = core // GROUPS
        y[b] += res.results[core]["yT"].T
    return y


if __name__ == "__main__":
    rng = np.random.default_rng(0)
    ins = {
        "x": rng.standard_normal((2, L, D)).astype(np.float32),
        "Wq": (0.02 * rng.standard_normal((D, D))).astype(np.float32),
        "Wk": (0.02 * rng.standard_normal((D, D))).astype(np.float32),
        "Wv": (0.02 * rng.standard_normal((D, D))).astype(np.float32),
        "Wo": (0.02 * rng.standard_normal((D, D))).astype(np.float32),
        "Wb": (0.02 * rng.standard_normal((D, H))).astype(np.float32),
        "bb": np.zeros(H, dtype=np.float32),
    }
    out = kernel(**ins)
    print("kernel ran, out shape", out.shape, "mean abs", np.abs(out).mean())



# revision 5
# speedup vs baseline: 1.2134x; 1.2134x over previous
"""GatedDeltaNet attention kernel for 8 Trainium2 NeuronCores.

Problem: B=2, L=2048, D=1024, H=16 heads (Dh=64).
  q,k,v = x@Wq, x@Wk, x@Wv ; beta = sigmoid(x@Wb + bb)
  q,k l2-normalized per head; out[l] = sum_{t<=l} beta_t <qh_l,kh_t> vh_t
  y = out @ Wo

Sharding: 8 cores = 2 batches x 4 head-groups (4 heads each). Each core
computes its batch/heads slice end-to-end including a partial y (contraction
over its 256 Wo rows); host sums the 4 partials per batch.

Device algorithm (per core), bf16 matmul operands, f32 PSUM accumulation:
  P1: qT/kT = W^T projections into [d', l] layout (lhsT=W, rhs=xT), c-major
      so matmuls chase the x DMA chunks; per-head squared norms of q AND k
      land in one PSUM bank via [l,h]-layout selector matmuls, one
      Abs_reciprocal_sqrt produces all 1/|q|,1/|k| factors. v projection
      (beta logits fused as 4 extra columns) into [t, e]; 1/|k| and beta
      fold into v ("vtilde"). k in [t, d] layout for the state path comes
      from PE transposes of kT (not a second GEMM).
  P2: chunked DeltaNet: per chunk, score tiles ST[t,l] per (pair,head-half),
      causal diagonal handled by one mask-multiply on the PSUM->SBUF copy;
      out2 accumulated in [l, e] layout (64-wide free dim = half the PE
      cost of the [e, l] layout), inter-chunk state S applied the same way;
      1/|q| folds into the o2->attn copy; attnT recovered by PE transposes.
  P3: yT = Wo^T @ attnT per chunk, interleaved into the next chunk's P2 as
      PE filler; bf16 output halves the out-DMA and the final drain.
"""

import numpy as np

P = 128
L = 2048
D = 1024
H = 16
KS = D // P        # 8 contraction subtiles
NT = L // P        # 16 t-blocks
CH = 512
NCH = L // CH      # 4 l-chunks
DH = 64
HC = 4             # heads per core
NP = HC // 2       # head pairs per core
NCORES = 8
GROUPS = NCORES // 2  # head groups (4)
NV = HC * DH       # 256
NKN = NT - NT // NCH  # 12 t-blocks that enter the state

_CACHE = {}
DEBUG_DUMP = False


def _build_nc():
    import concourse.bass as bass  # noqa: F401
    import concourse.tile as tile
    import concourse.mybir as mybir
    from concourse import bacc
    from contextlib import ExitStack

    F32 = mybir.dt.float32
    F32R = mybir.dt.float32r
    BF16 = mybir.dt.bfloat16
    AF = mybir.ActivationFunctionType
    OP = mybir.AluOpType

    nc = bacc.Bacc(
        "TRN2", target_bir_lowering=False, debug=False, num_devices=NCORES
    )

    xT = nc.dram_tensor("xT", [KS, P, L], BF16, kind="ExternalInput")
    wq = nc.dram_tensor("wq", [KS, P, NP * P], BF16, kind="ExternalInput")
    wk = nc.dram_tensor("wk", [KS, P, NP * P], BF16, kind="ExternalInput")
    wvb = nc.dram_tensor("wvb", [KS, P, NV + HC], BF16, kind="ExternalInput")
    wo = nc.dram_tensor("wo", [NP, P, D], BF16, kind="ExternalInput")
    sel = nc.dram_tensor("sel", [P, 2], F32R, kind="ExternalInput")
    bbb = nc.dram_tensor("bbb", [P, HC], F32, kind="ExternalInput")
    masks = nc.dram_tensor("masks", [P, P], F32, kind="ExternalInput")
    eye = nc.dram_tensor("eye", [P, P], BF16, kind="ExternalInput")
    yT = nc.dram_tensor("yT", [D, L], BF16, kind="ExternalOutput")
    dbg = {}
    if DEBUG_DUMP:
        for nm, shp, dt_ in [
            ("d_qT0", [P, L], BF16), ("d_kT0", [P, L], BF16),
            ("d_vt", [P, NT * NV], BF16), ("d_kn", [P, NKN * NV], BF16),
            ("d_factor", [P, NT * HC], F32), ("d_rnkq", [P, NT * 8], F32),
            ("d_attnT0", [P, L], BF16),
        ]:
            dbg[nm] = nc.dram_tensor(nm, shp, dt_, kind="ExternalOutput")

    with tile.TileContext(nc) as tc:
        with ExitStack() as ctx:
            pconst = ctx.enter_context(tc.tile_pool(name="const", bufs=1))
            pmain = ctx.enter_context(tc.tile_pool(name="main", bufs=1))

            # ---- engine-load balancing for PSUM->SBUF copies / elementwise
            eng_load = {"dve": 0.0, "act": 0.0, "pool": 0.0}

            def cost_dve(fd, psum=True, all16=False):
                init = 120 if psum else 58
                return 1.0417 * (init / 2 + fd * (0.5 if all16 else 1.0)) + 70

            def cost_act(fd):
                return 0.8333 * (222 / 2 + fd) + 57

            def cost_pool(fd, mult=False):
                return 95 + 0.8333 * fd / (0.42 if mult else 0.6) + 61

            def pick(costs):
                # costs: list of (engine, cost); choose min accumulated
                best = min(costs, key=lambda ec: eng_load[ec[0]] + ec[1])
                eng_load[best[0]] += best[1]
                return best[0]

            def bal_copy(out_ap, in_ap, fd, all16=False, pool_ok=True):
                costs = [("dve", cost_dve(fd, all16=all16)),
                         ("act", cost_act(fd))]
                if pool_ok:
                    costs.append(("pool", cost_pool(fd)))
                e = pick(costs)
                if e == "dve":
                    nc.vector.tensor_copy(out_ap, in_ap)
                elif e == "act":
                    nc.scalar.activation(out_ap, in_ap, AF.Copy)
                else:
                    nc.gpsimd.tensor_copy(out_ap, in_ap)

            def bal_mult(out_ap, a_ap, b_ap, fd, pool_ok=True):
                costs = [("dve", cost_dve(fd))]
                if pool_ok:
                    costs.append(("pool", cost_pool(fd, mult=True)))
                e = pick(costs)
                if e == "dve":
                    nc.vector.tensor_tensor(out_ap, a_ap, b_ap, OP.mult)
                else:
                    nc.gpsimd.tensor_tensor(out_ap, a_ap, b_ap, OP.mult)

            # ---- constant tiles
            sel_sb = pconst.tile([P, 2], F32R, tag="sel", name="sel")
            bbb_sb = pconst.tile([P, HC], F32, tag="bbb", name="bbb")
            mask_sb = pconst.tile([P, P], F32, tag="mask", name="mask")
            eye_sb = pconst.tile([P, P], BF16, tag="eye", name="eye")
            wo_sb = pconst.tile([P, NP, D], BF16, tag="wo", name="wo")

            # ---- persistent SBUF tensors
            qT = [pmain.tile([P, L], BF16, tag=f"qT{p}", name=f"qT{p}")
                  for p in range(NP)]
            kT = [pmain.tile([P, L], BF16, tag=f"kT{p}", name=f"kT{p}")
                  for p in range(NP)]
            vt = pmain.tile([P, NT, NV], BF16, tag="vt", name="vt")
            kn = pmain.tile([P, NKN, NV], BF16, tag="kn", name="kn")
            factor = pmain.tile([P, NT, HC], F32, tag="factor", name="factor")
            # 1/|q| (cols 0:4 = pair*2+hh) and 1/|k| (cols 4:8), per t-block
            rnkq = pmain.tile([P, NT, 8], F32, tag="rnkq", name="rnkq")
            attnT = [pmain.tile([P, L], BF16, tag=f"attnT{p}", name=f"attnT{p}")
                     for p in range(NP)]
            s_sb = pmain.tile([P, NP, NV], BF16, tag="ssb", name="ssb")

            # weights + x
            wq_sb = pmain.tile([P, KS, NP * P], BF16, tag="wq", name="wq")
            wk_sb = pmain.tile([P, KS, NP * P], BF16, tag="wk", name="wk")
            wvb_sb = pmain.tile([P, KS, NV + HC], BF16, tag="wvb", name="wvb")
            x_sb = [pmain.tile([P, L], BF16, tag=f"x{ks}", name=f"x{ks}")
                    for ks in range(KS)]

            # ---- input DMAs, issue order == consumption order (SP queue).
            # HWDGE serializes ~625ns per trigger, so later x chunks use
            # coarser granularity to keep the trigger count low.
            nc.sync.dma_start(wq_sb[:], wq.ap().rearrange("k p c -> p k c"))
            for ks in range(KS):
                nc.sync.dma_start(
                    x_sb[ks][:, 0:CH], xT.ap()[ks][:, 0:CH])
            nc.sync.dma_start(sel_sb[:], sel.ap())
            nc.sync.dma_start(bbb_sb[:], bbb.ap())
            nc.sync.dma_start(wk_sb[:], wk.ap().rearrange("k p c -> p k c"))
            for ks in range(KS):
                nc.sync.dma_start(
                    x_sb[ks][:, CH:2 * CH], xT.ap()[ks][:, CH:2 * CH])
            nc.sync.dma_start(
                wvb_sb[:], wvb.ap().rearrange("k p c -> p k c"))
            for ks in range(KS):
                nc.sync.dma_start(
                    x_sb[ks][:, 2 * CH:L], xT.ap()[ks][:, 2 * CH:L])
            nc.sync.dma_start(mask_sb[:], masks.ap())
            nc.sync.dma_start(eye_sb[:], eye.ap())
            nc.sync.dma_start(
                wo_sb[:], wo.ap().rearrange("s p d -> p s d"))

            # ---- SBUF pools
            psq = ctx.enter_context(tc.tile_pool(name="sq", bufs=3))
            ptmp = ctx.enter_context(tc.tile_pool(name="tmp", bufs=4))
            pst = ctx.enter_context(tc.tile_pool(name="stbuf", bufs=8))
            pal = ctx.enter_context(tc.tile_pool(name="attnl", bufs=4))
            pyout = ctx.enter_context(tc.tile_pool(name="yout", bufs=6))

            # ---- PSUM pools (8 banks: ppA 3 + ppO 2 + ppS 1 + ppY 2;
            #      ppN's bank is only live during P1 before ppY is used)
            ppA = ctx.enter_context(
                tc.tile_pool(name="ppA", bufs=3, space="PSUM"))
            ppO = ctx.enter_context(
                tc.tile_pool(name="ppO", bufs=2, space="PSUM"))
            ppS = ctx.enter_context(
                tc.tile_pool(name="ppS", bufs=1, space="PSUM"))

            s_ps = ppS.tile([P, NP, NV], F32, tag="sps", name="sps")

            # ================= P1: q/k projections + norms =================
            nsel = [0]

            def issue_norm_matmuls(sq_ap, c, wi, pair):
                for tr in range(CH // P):
                    tb = c * (CH // P) + tr
                    col = wi * 4 + pair * 2
                    nc.tensor.matmul(
                        normbank[:, tb, col:col + 2],
                        sq_ap[:, tr * P:(tr + 1) * P],
                        sel_sb[:],
                        start=(nsel[0] == 0),
                        stop=(nsel[0] == 4 * NP * NCH - 1),
                        skip_group_check=True,
                    )
                    nsel[0] += 1

            with ExitStack() as pnctx:
                ppN = pnctx.enter_context(
                    tc.tile_pool(name="ppN", bufs=1, space="PSUM"))
                normbank = ppN.tile([P, NT, 8], F32, tag="nb", name="nb")

                for c in range(NCH):
                    cs = slice(c * CH, (c + 1) * CH)
                    for wi, w_sb, dstT in ((0, wq_sb, qT), (1, wk_sb, kT)):
                        ps = [ppA.tile([P, CH], F32, tag="mm", name="mm")
                              for _ in range(NP)]
                        for ks in range(KS):
                            for pair in range(NP):
                                nc.tensor.matmul(
                                    ps[pair][:],
                                    w_sb[:, ks, pair * P:(pair + 1) * P],
                                    x_sb[ks][:, cs],
                                    start=(ks == 0),
                                    stop=(ks == KS - 1),
                                )
                        for pair in range(NP):
                            bal_copy(dstT[pair][:, cs], ps[pair][:], CH)
                            sq = psq.tile([P, CH], F32R, tag="sq", name="sq")
                            # square on ACT or DVE, whichever is freer
                            cd, ca = cost_dve(CH), cost_act(CH)
                            if eng_load["dve"] + cd <= eng_load["act"] + ca:
                                eng_load["dve"] += cd
                                nc.vector.tensor_tensor(
                                    sq[:], ps[pair][:], ps[pair][:], OP.mult)
                            else:
                                eng_load["act"] += ca
                                nc.scalar.activation(
                                    sq[:], ps[pair][:], AF.Square)
                            issue_norm_matmuls(sq, c, wi, pair)

                # all 64 selector matmuls done -> one activation for all norms
                nc.scalar.activation(
                    rnkq[:].rearrange("p a b -> p (a b)"),
                    normbank[:].rearrange("p a b -> p (a b)"),
                    AF.Abs_reciprocal_sqrt)
                eng_load["act"] += cost_act(NT * 8)

            # ppY created after ppN closed: peak PSUM stays at 8 banks
            ppY = ctx.enter_context(
                tc.tile_pool(name="ppY", bufs=2, space="PSUM"))

            # ---------------- v projection / kn transpose items ------------
            def v_item(tb):
                def run():
                    psv = ppA.tile([P, NV + HC], F32, tag="mm", name="mmv")
                    for ks in range(KS):
                        nc.tensor.matmul(
                            psv[:],
                            x_sb[ks][:, tb * P:(tb + 1) * P],
                            wvb_sb[:, ks, :],
                            start=(ks == 0),
                            stop=(ks == KS - 1),
                        )
                    bl = ptmp.tile([P, HC], F32, tag="bl", name="bl")
                    nc.vector.tensor_tensor(
                        bl[:], psv[:, NV:], bbb_sb[:], OP.add)
                    eng_load["dve"] += cost_dve(HC)
                    bs = ptmp.tile([P, HC], F32, tag="bs", name="bs")
                    nc.scalar.activation(bs[:], bl[:], AF.Sigmoid)
                    eng_load["act"] += cost_act(HC)
                    nc.vector.tensor_tensor(
                        factor[:, tb, :], bs[:], rnkq[:, tb, 4:8], OP.mult)
                    eng_load["dve"] += cost_dve(HC, psum=False)
                    bal_mult(
                        vt[:, tb, :].rearrange("p (h e) -> p h e", e=DH),
                        psv[:, :NV].rearrange("p (h e) -> p h e", e=DH),
                        factor[:, tb, :, None].to_broadcast((P, HC, DH)),
                        NV, pool_ok=False)
                return run

            def kn_item(tb, pair):
                def run():
                    trp = ppA.tile([P, P], BF16, tag="mm", name="mmt")
                    nc.tensor.matmul(
                        trp[:],
                        kT[pair][:, tb * P:(tb + 1) * P],
                        eye_sb[:],
                        is_transpose=True,
                    )
                    bal_copy(kn[:, tb, pair * P:(pair + 1) * P], trp[:],
                             P, all16=True)
                return run

            # ---------------- P3 items (yT chunk output) -------------------
            def p3_item(c, m, half=None):
                def run():
                    if half is None:
                        lo, hi = 0, CH
                    else:
                        lo, hi = half * (CH // 2), (half + 1) * (CH // 2)
                    w = hi - lo
                    py = ppY.tile([P, CH // 2 if half is not None else CH],
                                  F32, tag="py", name="py")
                    for pair in range(NP):
                        nc.tensor.matmul(
                            py[:, :w],
                            wo_sb[:, pair, m * P:(m + 1) * P],
                            attnT[pair][:, c * CH + lo:c * CH + hi],
                            start=(pair == 0),
                            stop=(pair == NP - 1),
                        )
                    yo = pyout.tile([P, CH], BF16, tag="yo", name="yo")
                    bal_copy(yo[:, :w], py[:, :w], w)
                    nc.sync.dma_start(
                        yT.ap()[m * P:(m + 1) * P, c * CH + lo:c * CH + hi],
                        yo[:, :w])
                return run

            # eager: v+kn for chunk 0 (needed by P2 c0 / fold at c1)
            for tb in range(4):
                v_item(tb)()
            for tb in range(4):
                for pair in range(NP):
                    kn_item(tb, pair)()

            # filler queues per P2 chunk
            fillers = {c: [] for c in range(NCH)}
            for tb in range(4, 8):
                fillers[0].append(v_item(tb))
            for tb in range(4, 8):
                for pair in range(NP):
                    fillers[0].append(kn_item(tb, pair))
            for tb in range(8, 12):
                fillers[1].append(v_item(tb))
            for tb in range(8, 12):
                for pair in range(NP):
                    fillers[1].append(kn_item(tb, pair))
            for tb in range(12, 16):
                fillers[2].append(v_item(tb))
            for c in range(1, NCH):
                for m in range(D // P):
                    fillers[c].append(p3_item(c - 1, m))

            def pop_fill(c, n):
                for _ in range(n):
                    if fillers[c]:
                        fillers[c].pop(0)()

            # ================= P2: chunked DeltaNet =================
            for c in range(NCH):
                o2l = {}
                for pair in range(NP):
                    o2l[pair] = ppO.tile([P, NCH, P], F32, tag="o2l",
                                         name=f"o2l{pair}")
                if c > 0:
                    # fold chunk c-1 into the state, snapshot to bf16
                    for pair in range(NP):
                        for tsub in range(4):
                            tb = (c - 1) * 4 + tsub
                            nc.tensor.matmul(
                                s_ps[:, pair, :],
                                kn[:, tb, pair * P:(pair + 1) * P],
                                vt[:, tb, :],
                                start=(c == 1 and tsub == 0),
                                stop=(c == NCH - 1 and tsub == 3),
                                skip_group_check=True,
                            )
                        bal_copy(s_sb[:, pair, :], s_ps[:, pair, :], NV)
                    pop_fill(c, 2)
                    # inter-chunk: o2l[lb, hh*64:..] = qT^T @ S  (free dim 64)
                    for pair in range(NP):
                        for hh in range(2):
                            h = 2 * pair + hh
                            for lb in range(NCH):
                                nc.tensor.matmul(
                                    o2l[pair][:, lb, hh * DH:(hh + 1) * DH],
                                    qT[pair][
                                        64 * hh:64 * (hh + 1),
                                        c * CH + lb * P:c * CH + (lb + 1) * P,
                                    ],
                                    s_sb[64 * hh:64 * (hh + 1), pair,
                                         h * DH:(h + 1) * DH],
                                    start=True, stop=False,
                                    skip_group_check=True,
                                )
                for T in range(4 * c, 4 * c + 4):
                    j = T - 4 * c
                    lo = P * j
                    # score tiles ST[t, l] for both pairs/hh
                    stps = {}
                    for pair in range(NP):
                        for hh in range(2):
                            sp = ppA.tile([P, CH], F32, tag="mm", name="st")
                            nc.tensor.matmul(
                                sp[:, lo:CH],
                                kT[pair][64 * hh:64 * (hh + 1),
                                         T * P:(T + 1) * P],
                                qT[pair][64 * hh:64 * (hh + 1),
                                         c * CH + lo:(c + 1) * CH],
                                start=True, stop=True,
                            )
                            stps[(pair, hh)] = sp
                    pop_fill(c, 2)
                    st_sb = {}
                    for pair in range(NP):
                        for hh in range(2):
                            sb = pst.tile([P, CH], BF16, tag="st_sb",
                                          name="st_sb")
                            # triangular block at the causal frontier
                            bal_mult(sb[:, lo:lo + P],
                                     stps[(pair, hh)][:, lo:lo + P],
                                     mask_sb[:], P)
                            if lo + P < CH:
                                bal_copy(sb[:, lo + P:CH],
                                         stps[(pair, hh)][:, lo + P:CH],
                                         CH - lo - P)
                            st_sb[(pair, hh)] = sb
                    pop_fill(c, 2)
                    # out2 in [l, e]: o2l[lb] += ST[:, lb]^T @ vtilde
                    for pair in range(NP):
                        for hh in range(2):
                            h = 2 * pair + hh
                            for lb in range(j, NCH):
                                nc.tensor.matmul(
                                    o2l[pair][:, lb, hh * DH:(hh + 1) * DH],
                                    st_sb[(pair, hh)][:, lb * P:(lb + 1) * P],
                                    vt[:, T, h * DH:(h + 1) * DH],
                                    start=(c == 0 and j == 0),
                                    stop=(j == lb),
                                    skip_group_check=True,
                                )
                # o2l -> attn_l (fold in 1/|q|), then PE-transpose to attnT;
                # both mults issued first so the transposes' PSUM-slot reuse
                # never makes PE wait on a not-yet-issued DVE op
                als = []
                for pair in range(NP):
                    al = pal.tile([P, NCH, P], BF16, tag="al", name="al")
                    bal_mult(
                        al[:].rearrange("p a (b e) -> p a b e", e=DH),
                        o2l[pair][:].rearrange("p a (b e) -> p a b e", e=DH),
                        rnkq[:, 4 * c:4 * (c + 1), 2 * pair:2 * pair + 2, None]
                        .to_broadcast((P, NCH, 2, DH)),
                        NCH * P, pool_ok=False)
                    als.append(al)
                pop_fill(c, 2)
                for pair in range(NP):
                    for lb in range(NCH):
                        trp = ppO.tile([P, P], BF16, tag="o2l", name="altr")
                        nc.tensor.matmul(
                            trp[:], als[pair][:, lb, :], eye_sb[:],
                            is_transpose=True,
                        )
                        bal_copy(
                            attnT[pair][:, c * CH + lb * P:
                                        c * CH + (lb + 1) * P],
                            trp[:], P, all16=True)
                pop_fill(c, 99)

            # last chunk's P3 in half-column granularity for a short drain
            for m in range(D // P):
                for half in range(2):
                    p3_item(NCH - 1, m, half=half)()

            if DEBUG_DUMP:
                nc.sync.dma_start(dbg["d_qT0"].ap(), qT[0][:])
                nc.sync.dma_start(dbg["d_kT0"].ap(), kT[0][:])
                nc.sync.dma_start(
                    dbg["d_vt"].ap(), vt[:].rearrange("p a b -> p (a b)"))
                nc.sync.dma_start(
                    dbg["d_kn"].ap(), kn[:].rearrange("p a b -> p (a b)"))
                nc.sync.dma_start(
                    dbg["d_factor"].ap(),
                    factor[:].rearrange("p a b -> p (a b)"))
                nc.sync.dma_start(
                    dbg["d_rnkq"].ap(),
                    rnkq[:].rearrange("p a b -> p (a b)"))
                nc.sync.dma_start(dbg["d_attnT0"].ap(), attnT[0][:])

    nc.compile()
    return nc


def get_nc():
    if "nc" not in _CACHE:
        _CACHE["nc"] = _build_nc()
    return _CACHE["nc"]


def make_core_inputs(x, Wq, Wk, Wv, Wo, Wb, bb):
    """Build the 8 per-core input maps from full inputs."""
    import ml_dtypes
    BF = ml_dtypes.bfloat16

    x = np.asarray(x, dtype=np.float32)
    Wq = np.asarray(Wq, dtype=np.float32)
    Wk = np.asarray(Wk, dtype=np.float32)
    Wv = np.asarray(Wv, dtype=np.float32)
    Wo = np.asarray(Wo, dtype=np.float32)
    Wb = np.asarray(Wb, dtype=np.float32)
    bb = np.asarray(bb, dtype=np.float32)

    selm = np.zeros((P, 2), dtype=np.float32)
    selm[:64, 0] = 1.0
    selm[64:, 1] = 1.0
    maskm = (np.arange(P)[:, None] <= np.arange(P)[None, :]).astype(np.float32)
    eyem = np.eye(P, dtype=BF)

    in_maps = []
    for core in range(NCORES):
        b, g = divmod(core, GROUPS)
        hs = slice(NV * g, NV * (g + 1))
        bs = slice(HC * g, HC * (g + 1))
        xTc = np.ascontiguousarray(x[b].T).reshape(KS, P, L).astype(BF)
        wqc = np.ascontiguousarray(Wq[:, hs]).reshape(KS, P, NP * P).astype(BF)
        wkc = np.ascontiguousarray(Wk[:, hs]).reshape(KS, P, NP * P).astype(BF)
        wvbc = np.ascontiguousarray(
            np.concatenate([Wv[:, hs], Wb[:, bs]], axis=1)
        ).reshape(KS, P, NV + HC).astype(BF)
        woc = np.ascontiguousarray(Wo[hs, :]).reshape(NP, P, D).astype(BF)
        bbbc = np.ascontiguousarray(np.tile(bb[bs][None, :], (P, 1)))
        in_maps.append(
            {
                "xT": xTc,
                "wq": wqc,
                "wk": wkc,
                "wvb": wvbc,
                "wo": woc,
                "sel": selm,
                "bbb": bbbc,
                "masks": maskm,
                "eye": eyem,
            }
        )
    return in_maps


def kernel(x, Wq, Wk, Wv, Wo, Wb, bb):
    from concourse.bass_utils import run_bass_kernel_spmd

    nc = get_nc()
    in_maps = make_core_inputs(x, Wq, Wk, Wv, Wo, Wb, bb)
    try:
        res = run_bass_kernel_spmd(nc, in_maps, core_ids=list(range(NCORES)))
    except Exception:
        # transient NRT wedges (e.g. NRT_EXEC_UNIT_UNRECOVERABLE) clear on
        # a fresh attempt; retry once before giving up
        res = run_bass_kernel_spmd(nc, in_maps, core_ids=list(range(NCORES)))
    B = 2
    y = np.zeros((B, L, D), dtype=np.float32)
    for core in range(NCORES):
        b = core // GROUPS
        y[b] += np.asarray(res.results[core]["yT"]).astype(np.float32).T
    return y


if __name__ == "__main__":
    rng = np.random.default_rng(0)
    ins = {
        "x": rng.standard_normal((2, L, D)).astype(np.float32),
        "Wq": (0.02 * rng.standard_normal((D, D))).astype(np.float32),
        "Wk": (0.02 * rng.standard_normal((D, D))).astype(np.float32),
        "Wv": (0.02 * rng.standard_normal((D, D))).astype(np.float32),
        "Wo": (0.02 * rng.standard_normal((D, D))).astype(np.float32),
        "Wb": (0.02 * rng.standard_normal((D, H))).astype(np.float32),
        "bb": np.zeros(H, dtype=np.float32),
    }
    out = kernel(**ins)
    print("kernel ran, out shape", out.shape, "mean abs", np.abs(out).mean())


# revision 19
# speedup vs baseline: 1.2226x; 1.0076x over previous
"""GatedDeltaNet attention kernel for 8 Trainium2 NeuronCores.

Problem: B=2, L=2048, D=1024, H=16 heads (Dh=64).
  q,k,v = x@Wq, x@Wk, x@Wv ; beta = sigmoid(x@Wb + bb)
  q,k l2-normalized per head; out[l] = sum_{t<=l} beta_t <qh_l,kh_t> vh_t
  y = out @ Wo

Sharding: 8 cores = 2 batches x 4 head-groups (4 heads each). Each core
computes its batch/heads slice end-to-end including a partial y (contraction
over its 256 Wo rows); host sums the 4 partials per batch.

Device algorithm (per core), bf16 matmul operands, f32 PSUM accumulation:
  P1: qT/kT = W^T projections into [d', l] layout (lhsT=W, rhs=xT), c-major
      so matmuls chase the x DMA chunks; per-head squared norms of q AND k
      land in one PSUM bank via [l,h]-layout selector matmuls, one
      Abs_reciprocal_sqrt produces all 1/|q|,1/|k| factors. v projection
      (beta logits fused as 4 extra columns) into [t, e]; 1/|k| and beta
      fold into v ("vtilde"). k in [t, d] layout for the state path comes
      from PE transposes of kT (not a second GEMM).
  P2: chunked DeltaNet: per chunk, score tiles ST[t,l] per (pair,head-half),
      causal diagonal handled by one mask-multiply on the PSUM->SBUF copy;
      out2 accumulated in [l, e] layout (64-wide free dim = half the PE
      cost of the [e, l] layout), inter-chunk state S applied the same way;
      1/|q| folds into the o2->attn copy; attnT recovered by PE transposes.
  P3: yT = Wo^T @ attnT per chunk, interleaved into the next chunk's P2 as
      PE filler; bf16 output halves the out-DMA and the final drain.
"""

import numpy as np

P = 128
L = 2048
D = 1024
H = 16
KS = D // P        # 8 contraction subtiles
NT = L // P        # 16 t-blocks
CH = 512
NCH = L // CH      # 4 l-chunks
DH = 64
HC = 4             # heads per core
NP = HC // 2       # head pairs per core
NCORES = 8
GROUPS = NCORES // 2  # head groups (4)
NV = HC * DH       # 256
NKN = NT - NT // NCH  # 12 t-blocks that enter the state

_CACHE = {}
DEBUG_DUMP = False


def _build_nc():
    import concourse.bass as bass  # noqa: F401
    import concourse.tile as tile
    import concourse.mybir as mybir
    from concourse import bacc
    from contextlib import ExitStack

    F32 = mybir.dt.float32
    F32R = mybir.dt.float32r
    BF16 = mybir.dt.bfloat16
    AF = mybir.ActivationFunctionType
    OP = mybir.AluOpType

    nc = bacc.Bacc(
        "TRN2", target_bir_lowering=False, debug=False, num_devices=NCORES
    )

    xT = nc.dram_tensor("xT", [KS, P, L], BF16, kind="ExternalInput")
    wq = nc.dram_tensor("wq", [KS, P, NP * P], BF16, kind="ExternalInput")
    wk = nc.dram_tensor("wk", [KS, P, NP * P], BF16, kind="ExternalInput")
    wv = nc.dram_tensor("wv", [KS, P, NV], BF16, kind="ExternalInput")
    wb = nc.dram_tensor("wb", [KS, P, HC], BF16, kind="ExternalInput")
    wo = nc.dram_tensor("wo", [NP, P, D], BF16, kind="ExternalInput")
    sel = nc.dram_tensor("sel", [P, 2], F32R, kind="ExternalInput")
    bbb = nc.dram_tensor("bbb", [P, HC], F32, kind="ExternalInput")
    masks = nc.dram_tensor("masks", [P, P], F32, kind="ExternalInput")
    eye = nc.dram_tensor("eye", [P, P], BF16, kind="ExternalInput")
    yT = nc.dram_tensor("yT", [D, L], BF16, kind="ExternalOutput")
    dbg = {}
    if DEBUG_DUMP:
        for nm, shp, dt_ in [
            ("d_qT0", [P, L], BF16), ("d_kT0", [P, L], BF16),
            ("d_vt", [P, NT * NV], BF16), ("d_kn", [P, NKN * NV], BF16),
            ("d_factor", [P, NT * HC], F32), ("d_rnkq", [P, NT * 8], F32),
            ("d_attnT0", [P, L], BF16),
        ]:
            dbg[nm] = nc.dram_tensor(nm, shp, dt_, kind="ExternalOutput")

    with tile.TileContext(nc) as tc:
        with ExitStack() as ctx:
            pconst = ctx.enter_context(tc.tile_pool(name="const", bufs=1))
            pmain = ctx.enter_context(tc.tile_pool(name="main", bufs=1))

            # ---- engine-load balancing for PSUM->SBUF copies / elementwise
            eng_load = {"dve": 0.0, "act": 0.0, "pool": 0.0}

            def cost_dve(fd, psum=True, all16=False):
                init = 120 if psum else 58
                return 1.0417 * (init / 2 + fd * (0.5 if all16 else 1.0)) + 70

            def cost_act(fd):
                return 0.8333 * (222 / 2 + fd) + 57

            def cost_pool(fd, mult=False):
                return 95 + 0.8333 * fd / (0.42 if mult else 0.6) + 61

            def pick(costs):
                # costs: list of (engine, cost); choose min accumulated
                best = min(costs, key=lambda ec: eng_load[ec[0]] + ec[1])
                eng_load[best[0]] += best[1]
                return best[0]

            def bal_copy(out_ap, in_ap, fd, all16=False, pool_ok=True):
                costs = [("dve", cost_dve(fd, all16=all16)),
                         ("act", cost_act(fd))]
                if pool_ok:
                    costs.append(("pool", cost_pool(fd)))
                e = pick(costs)
                if e == "dve":
                    nc.vector.tensor_copy(out_ap, in_ap)
                elif e == "act":
                    nc.scalar.activation(out_ap, in_ap, AF.Copy)
                else:
                    nc.gpsimd.tensor_copy(out_ap, in_ap)

            def bal_mult(out_ap, a_ap, b_ap, fd, pool_ok=True):
                costs = [("dve", cost_dve(fd))]
                if pool_ok:
                    costs.append(("pool", cost_pool(fd, mult=True)))
                e = pick(costs)
                if e == "dve":
                    nc.vector.tensor_tensor(out_ap, a_ap, b_ap, OP.mult)
                else:
                    nc.gpsimd.tensor_tensor(out_ap, a_ap, b_ap, OP.mult)

            # ---- constant tiles
            sel_sb = pconst.tile([P, 2], F32R, tag="sel", name="sel")
            bbb_sb = pconst.tile([P, HC], F32, tag="bbb", name="bbb")
            mask_sb = pconst.tile([P, P], F32, tag="mask", name="mask")
            eye_sb = pconst.tile([P, P], BF16, tag="eye", name="eye")
            wo_sb = pconst.tile([P, NP, D], BF16, tag="wo", name="wo")

            # ---- persistent SBUF tensors
            qT = [pmain.tile([P, L], BF16, tag=f"qT{p}", name=f"qT{p}")
                  for p in range(NP)]
            kT = [pmain.tile([P, L], BF16, tag=f"kT{p}", name=f"kT{p}")
                  for p in range(NP)]
            vt = pmain.tile([P, NT, NV], BF16, tag="vt", name="vt")
            kn = pmain.tile([P, NKN, NV], BF16, tag="kn", name="kn")
            factor = pmain.tile([P, NT, HC], F32, tag="factor", name="factor")
            # 1/|q| (cols 0:4 = pair*2+hh) and 1/|k| (cols 4:8), per t-block
            rnkq = pmain.tile([P, NT, 8], F32, tag="rnkq", name="rnkq")
            attnT = [pmain.tile([P, L], BF16, tag=f"attnT{p}", name=f"attnT{p}")
                     for p in range(NP)]
            s_sb = pmain.tile([P, NP, NV], BF16, tag="ssb", name="ssb")

            # weights + x
            wq_sb = pmain.tile([P, KS, NP * P], BF16, tag="wq", name="wq")
            wk_sb = pmain.tile([P, KS, NP * P], BF16, tag="wk", name="wk")
            wv_sb = pmain.tile([P, KS, NV], BF16, tag="wv", name="wv")
            wb_sb = pmain.tile([P, KS, HC], BF16, tag="wb", name="wb")
            x_sb = [pmain.tile([P, L], BF16, tag=f"x{ks}", name=f"x{ks}")
                    for ks in range(KS)]

            # ---- input DMAs, issue order == consumption order (SP queue).
            # HWDGE serializes ~625ns per trigger, so later x chunks use
            # coarser granularity to keep the trigger count low.
            nc.sync.dma_start(wq_sb[:, 0:KS // 2, :],
                              wq.ap()[0:KS // 2].rearrange("k p c -> p k c"))
            for ks in range(KS):
                nc.sync.dma_start(
                    x_sb[ks][:, 0:CH], xT.ap()[ks][:, 0:CH])
                if ks == 1:
                    nc.sync.dma_start(
                        wq_sb[:, KS // 2:, :],
                        wq.ap()[KS // 2:].rearrange("k p c -> p k c"))
            nc.sync.dma_start(sel_sb[:], sel.ap())
            nc.sync.dma_start(bbb_sb[:], bbb.ap())
            nc.sync.dma_start(wk_sb[:], wk.ap().rearrange("k p c -> p k c"))
            for ks in range(KS):
                nc.sync.dma_start(
                    x_sb[ks][:, CH:2 * CH], xT.ap()[ks][:, CH:2 * CH])
            nc.sync.dma_start(wv_sb[:], wv.ap().rearrange("k p c -> p k c"))
            nc.sync.dma_start(wb_sb[:], wb.ap().rearrange("k p c -> p k c"))
            for ks in range(KS):
                nc.sync.dma_start(
                    x_sb[ks][:, 2 * CH:L], xT.ap()[ks][:, 2 * CH:L])
            nc.sync.dma_start(mask_sb[:], masks.ap())
            nc.sync.dma_start(eye_sb[:], eye.ap())
            nc.sync.dma_start(
                wo_sb[:], wo.ap().rearrange("s p d -> p s d"))

            # ---- SBUF pools
            psq = ctx.enter_context(tc.tile_pool(name="sq", bufs=3))
            ptmp = ctx.enter_context(tc.tile_pool(name="tmp", bufs=4))
            pst = ctx.enter_context(tc.tile_pool(name="stbuf", bufs=8))
            pal = ctx.enter_context(tc.tile_pool(name="attnl", bufs=4))
            pyout = ctx.enter_context(tc.tile_pool(name="yout", bufs=6))

            # ---- PSUM pools (8 banks: ppA 3 + ppO 2 + ppS 1 + ppY 2;
            #      ppN's bank is only live during P1 before ppY is used)
            ppA = ctx.enter_context(
                tc.tile_pool(name="ppA", bufs=3, space="PSUM"))
            ppO = ctx.enter_context(
                tc.tile_pool(name="ppO", bufs=2, space="PSUM"))
            ppS = ctx.enter_context(
                tc.tile_pool(name="ppS", bufs=1, space="PSUM"))

            s_ps = ppS.tile([P, NP, NV], F32, tag="sps", name="sps")

            # ================= P1: q/k projections + norms =================
            nsel = [0]

            def issue_norm_matmuls(sq_ap, c, wi, pair):
                for tr in range(CH // P):
                    tb = c * (CH // P) + tr
                    col = wi * 4 + pair * 2
                    nc.tensor.matmul(
                        normbank[:, tb, col:col + 2],
                        sq_ap[:, tr * P:(tr + 1) * P],
                        sel_sb[:],
                        start=(nsel[0] == 0),
                        stop=(nsel[0] == 4 * NP * NCH - 1),
                        skip_group_check=True,
                    )
                    nsel[0] += 1

            with ExitStack() as pnctx:
                ppN = pnctx.enter_context(
                    tc.tile_pool(name="ppN", bufs=1, space="PSUM"))
                normbank = ppN.tile([P, NT, 8], F32, tag="nb", name="nb")
                betabank = ppN.tile([P, NT, HC], F32, tag="bb", name="bb")

                for c in range(NCH):
                    cs = slice(c * CH, (c + 1) * CH)
                    for wi, w_sb, dstT in ((0, wq_sb, qT), (1, wk_sb, kT)):
                        ps = [ppA.tile([P, CH], F32, tag="mm", name="mm")
                              for _ in range(NP)]
                        for ks in range(KS):
                            for pair in range(NP):
                                nc.tensor.matmul(
                                    ps[pair][:],
                                    w_sb[:, ks, pair * P:(pair + 1) * P],
                                    x_sb[ks][:, cs],
                                    start=(ks == 0),
                                    stop=(ks == KS - 1),
                                )
                        for pair in range(NP):
                            bal_copy(dstT[pair][:, cs], ps[pair][:], CH)
                            sq = psq.tile([P, CH], F32R, tag="sq", name="sq")
                            # square on ACT or DVE, whichever is freer
                            cd, ca = cost_dve(CH), cost_act(CH)
                            if eng_load["dve"] + cd <= eng_load["act"] + ca:
                                eng_load["dve"] += cd
                                nc.vector.tensor_tensor(
                                    sq[:], ps[pair][:], ps[pair][:], OP.mult)
                            else:
                                eng_load["act"] += ca
                                nc.scalar.activation(
                                    sq[:], ps[pair][:], AF.Square)
                            issue_norm_matmuls(sq, c, wi, pair)

                # beta logits GEMM (batched: all 16 t-blocks into one bank)
                for tb in range(NT):
                    for ks in range(KS):
                        nc.tensor.matmul(
                            betabank[:, tb, :],
                            x_sb[ks][:, tb * P:(tb + 1) * P],
                            wb_sb[:, ks, :],
                            start=(ks == 0),
                            stop=(ks == KS - 1),
                            skip_group_check=True,
                        )

                # all 64 selector matmuls done -> one activation for all
                # norms; beta chain batched so ACT needs only two table
                # loads (abs_recip + sigmoid), both hidden under v-phase PE
                nc.scalar.activation(
                    rnkq[:].rearrange("p a b -> p (a b)"),
                    normbank[:].rearrange("p a b -> p (a b)"),
                    AF.Abs_reciprocal_sqrt)
                eng_load["act"] += cost_act(NT * 8)
                bl_all = pmain.tile([P, NT, HC], F32, tag="bl", name="bl")
                nc.vector.tensor_tensor(
                    bl_all[:],
                    betabank[:],
                    bbb_sb[:, None, :].to_broadcast((P, NT, HC)),
                    OP.add)
                eng_load["dve"] += cost_dve(NT * HC)
                bs_all = pmain.tile([P, NT, HC], F32, tag="bs", name="bs")
                nc.scalar.activation(
                    bs_all[:].rearrange("p a b -> p (a b)"),
                    bl_all[:].rearrange("p a b -> p (a b)"),
                    AF.Sigmoid)
                eng_load["act"] += cost_act(NT * HC)
                nc.vector.tensor_tensor(
                    factor[:], bs_all[:], rnkq[:, :, 4:8], OP.mult)
                eng_load["dve"] += cost_dve(NT * HC, psum=False)

            # ppY created after ppN closed: peak PSUM stays at 8 banks
            ppY = ctx.enter_context(
                tc.tile_pool(name="ppY", bufs=2, space="PSUM"))

            # ---------------- v projection / kn transpose items ------------
            def v_item(tb):
                def run():
                    psv = ppA.tile([P, NV], F32, tag="mm", name="mmv")
                    for ks in range(KS):
                        nc.tensor.matmul(
                            psv[:],
                            x_sb[ks][:, tb * P:(tb + 1) * P],
                            wv_sb[:, ks, :],
                            start=(ks == 0),
                            stop=(ks == KS - 1),
                        )
                    bal_mult(
                        vt[:, tb, :].rearrange("p (h e) -> p h e", e=DH),
                        psv[:].rearrange("p (h e) -> p h e", e=DH),
                        factor[:, tb, :, None].to_broadcast((P, HC, DH)),
                        NV, pool_ok=False)
                return run

            def kn_item(tb, pair):
                def run():
                    trp = ppA.tile([P, P], BF16, tag="mm", name="mmt")
                    nc.tensor.matmul(
                        trp[:],
                        kT[pair][:, tb * P:(tb + 1) * P],
                        eye_sb[:],
                        is_transpose=True,
                    )
                    bal_copy(kn[:, tb, pair * P:(pair + 1) * P], trp[:],
                             P, all16=True)
                return run

            # ---------------- P3 items (yT chunk output) -------------------
            # yo tiles hold 4 m-blocks; one DMA writes [512, 512] of yT
            # (HWDGE triggers are 625ns each, so merge aggressively)
            yo_half = {}

            def p3_item(c, m):
                def run():
                    py = ppY.tile([P, CH], F32, tag="py", name="py")
                    for pair in range(NP):
                        nc.tensor.matmul(
                            py[:],
                            wo_sb[:, pair, m * P:(m + 1) * P],
                            attnT[pair][:, c * CH:(c + 1) * CH],
                            start=(pair == 0),
                            stop=(pair == NP - 1),
                        )
                    half, mi = divmod(m, 4)
                    if mi == 0:
                        yo_half[(c, half)] = pyout.tile(
                            [P, 4, CH], BF16, tag="yo", name="yo")
                    yo = yo_half[(c, half)]
                    bal_copy(yo[:, mi, :], py[:], CH)
                    if mi == 3:
                        nc.sync.dma_start(
                            yT.ap()[half * 4 * P:(half + 1) * 4 * P,
                                    c * CH:(c + 1) * CH]
                            .rearrange("(m p) c -> p m c", p=P),
                            yo[:],
                        )
                return run

            # eager: v+kn for chunk 0 (needed by P2 c0 / fold at c1)
            for tb in range(4):
                v_item(tb)()
            for tb in range(4):
                for pair in range(NP):
                    kn_item(tb, pair)()

            # filler queues per P2 chunk; popped evenly across the chunk's
            # fill points so late T-iterations don't starve
            fillers = {c: [] for c in range(NCH)}
            fillpts = {c: 16 for c in range(NCH)}
            for tb in range(4, 8):
                fillers[0].append(v_item(tb))
            for tb in range(4, 8):
                for pair in range(NP):
                    fillers[0].append(kn_item(tb, pair))
            for tb in range(8, 12):
                fillers[1].append(v_item(tb))
            for tb in range(8, 12):
                for pair in range(NP):
                    fillers[1].append(kn_item(tb, pair))
            for tb in range(12, 16):
                fillers[2].append(v_item(tb))
            for c in range(1, NCH):
                for m in range(D // P):
                    fillers[c].append(p3_item(c - 1, m))

            def pop_fill(c, pts=1):
                # consume a fair share of the remaining fillers
                n = -(-len(fillers[c]) // max(fillpts[c], 1)) * pts
                fillpts[c] = max(fillpts[c] - pts, 0)
                for _ in range(n):
                    if fillers[c]:
                        fillers[c].pop(0)()

            # ================= P2: chunked DeltaNet =================
            for c in range(NCH):
                o2l = {}
                for pair in range(NP):
                    o2l[pair] = ppO.tile([P, NCH, P], F32, tag="o2l",
                                         name=f"o2l{pair}")
                if c > 0:
                    # fold chunk c-1 into the state, snapshot to bf16
                    for pair in range(NP):
                        for tsub in range(4):
                            tb = (c - 1) * 4 + tsub
                            nc.tensor.matmul(
                                s_ps[:, pair, :],
                                kn[:, tb, pair * P:(pair + 1) * P],
                                vt[:, tb, :],
                                start=(c == 1 and tsub == 0),
                                stop=(c == NCH - 1 and tsub == 3),
                                skip_group_check=True,
                            )
                        bal_copy(s_sb[:, pair, :], s_ps[:, pair, :], NV,
                                 pool_ok=False)
                    pop_fill(c)
                    # inter-chunk: o2l[lb, hh*64:..] = qT^T @ S  (free dim 64)
                    for pair in range(NP):
                        for hh in range(2):
                            h = 2 * pair + hh
                            for lb in range(NCH):
                                nc.tensor.matmul(
                                    o2l[pair][:, lb, hh * DH:(hh + 1) * DH],
                                    qT[pair][
                                        64 * hh:64 * (hh + 1),
                                        c * CH + lb * P:c * CH + (lb + 1) * P,
                                    ],
                                    s_sb[64 * hh:64 * (hh + 1), pair,
                                         h * DH:(h + 1) * DH],
                                    start=True, stop=False,
                                    skip_group_check=True,
                                )
                for T in range(4 * c, 4 * c + 4):
                    j = T - 4 * c
                    lo = P * j
                    # score tiles ST[t, l] for both pairs/hh
                    stps = {}
                    for pair in range(NP):
                        for hh in range(2):
                            sp = ppA.tile([P, CH], F32, tag="mm", name="st")
                            nc.tensor.matmul(
                                sp[:, lo:CH],
                                kT[pair][64 * hh:64 * (hh + 1),
                                         T * P:(T + 1) * P],
                                qT[pair][64 * hh:64 * (hh + 1),
                                         c * CH + lo:(c + 1) * CH],
                                start=True, stop=True,
                            )
                            stps[(pair, hh)] = sp
                    pop_fill(c)
                    st_sb = {}
                    for pair in range(NP):
                        for hh in range(2):
                            sb = pst.tile([P, CH], BF16, tag="st_sb",
                                          name="st_sb")
                            # triangular block at the causal frontier
                            bal_mult(sb[:, lo:lo + P],
                                     stps[(pair, hh)][:, lo:lo + P],
                                     mask_sb[:], P, pool_ok=False)
                            if lo + P < CH:
                                bal_copy(sb[:, lo + P:CH],
                                         stps[(pair, hh)][:, lo + P:CH],
                                         CH - lo - P, pool_ok=False)
                            st_sb[(pair, hh)] = sb
                    pop_fill(c)
                    # out2 in [l, e]: o2l[lb] += ST[:, lb]^T @ vtilde
                    for pair in range(NP):
                        for hh in range(2):
                            h = 2 * pair + hh
                            for lb in range(j, NCH):
                                nc.tensor.matmul(
                                    o2l[pair][:, lb, hh * DH:(hh + 1) * DH],
                                    st_sb[(pair, hh)][:, lb * P:(lb + 1) * P],
                                    vt[:, T, h * DH:(h + 1) * DH],
                                    start=(c == 0 and j == 0),
                                    stop=(j == lb),
                                    skip_group_check=True,
                                )
                    pop_fill(c)
                # o2l -> attn_l (fold in 1/|q|) per l-block so the PE
                # transposes pipeline behind the multiplies
                als = {}
                for pair in range(NP):
                    als[pair] = pal.tile([P, NCH, P], BF16, tag="al",
                                         name="al")
                for lb in range(NCH):
                    for pair in range(NP):
                        ca2 = 2 * cost_act(DH)
                        cd2 = cost_dve(P)
                        if eng_load["act"] + ca2 <= eng_load["dve"] + cd2:
                            eng_load["act"] += ca2
                            for hh in range(2):
                                nc.scalar.activation(
                                    als[pair][:, lb, hh * DH:(hh + 1) * DH],
                                    o2l[pair][:, lb, hh * DH:(hh + 1) * DH],
                                    AF.Copy,
                                    scale=rnkq[:, 4 * c + lb,
                                               2 * pair + hh:
                                               2 * pair + hh + 1],
                                )
                        else:
                            eng_load["dve"] += cd2
                            nc.vector.tensor_tensor(
                                als[pair][:, lb, :]
                                .rearrange("p (b e) -> p b e", e=DH),
                                o2l[pair][:, lb, :]
                                .rearrange("p (b e) -> p b e", e=DH),
                                rnkq[:, 4 * c + lb,
                                     2 * pair:2 * pair + 2, None]
                                .to_broadcast((P, 2, DH)),
                                OP.mult)
                for pair in range(NP):
                    for lb in range(NCH):
                        trp = ppO.tile([P, P], BF16, tag="o2l", name="altr")
                        nc.tensor.matmul(
                            trp[:], als[pair][:, lb, :], eye_sb[:],
                            is_transpose=True,
                        )
                        bal_copy(
                            attnT[pair][:, c * CH + lb * P:
                                        c * CH + (lb + 1) * P],
                            trp[:], P, all16=True)
                    pop_fill(c)
                pop_fill(c, 2)

            # last chunk's P3
            for m in range(D // P):
                p3_item(NCH - 1, m)()

            if DEBUG_DUMP:
                nc.sync.dma_start(dbg["d_qT0"].ap(), qT[0][:])
                nc.sync.dma_start(dbg["d_kT0"].ap(), kT[0][:])
                nc.sync.dma_start(
                    dbg["d_vt"].ap(), vt[:].rearrange("p a b -> p (a b)"))
                nc.sync.dma_start(
                    dbg["d_kn"].ap(), kn[:].rearrange("p a b -> p (a b)"))
                nc.sync.dma_start(
                    dbg["d_factor"].ap(),
                    factor[:].rearrange("p a b -> p (a b)"))
                nc.sync.dma_start(
                    dbg["d_rnkq"].ap(),
                    rnkq[:].rearrange("p a b -> p (a b)"))
                nc.sync.dma_start(dbg["d_attnT0"].ap(), attnT[0][:])

    nc.compile()
    return nc


def get_nc():
    if "nc" not in _CACHE:
        _CACHE["nc"] = _build_nc()
    return _CACHE["nc"]


def make_core_inputs(x, Wq, Wk, Wv, Wo, Wb, bb):
    """Build the 8 per-core input maps from full inputs."""
    import ml_dtypes
    BF = ml_dtypes.bfloat16

    x = np.asarray(x, dtype=np.float32)
    Wq = np.asarray(Wq, dtype=np.float32)
    Wk = np.asarray(Wk, dtype=np.float32)
    Wv = np.asarray(Wv, dtype=np.float32)
    Wo = np.asarray(Wo, dtype=np.float32)
    Wb = np.asarray(Wb, dtype=np.float32)
    bb = np.asarray(bb, dtype=np.float32)

    selm = np.zeros((P, 2), dtype=np.float32)
    selm[:64, 0] = 1.0
    selm[64:, 1] = 1.0
    maskm = (np.arange(P)[:, None] <= np.arange(P)[None, :]).astype(np.float32)
    eyem = np.eye(P, dtype=BF)

    in_maps = []
    for core in range(NCORES):
        b, g = divmod(core, GROUPS)
        hs = slice(NV * g, NV * (g + 1))
        bs = slice(HC * g, HC * (g + 1))
        xTc = np.ascontiguousarray(x[b].T).reshape(KS, P, L).astype(BF)
        wqc = np.ascontiguousarray(Wq[:, hs]).reshape(KS, P, NP * P).astype(BF)
        wkc = np.ascontiguousarray(Wk[:, hs]).reshape(KS, P, NP * P).astype(BF)
        wvc = np.ascontiguousarray(Wv[:, hs]).reshape(KS, P, NV).astype(BF)
        wbc = np.ascontiguousarray(Wb[:, bs]).reshape(KS, P, HC).astype(BF)
        woc = np.ascontiguousarray(Wo[hs, :]).reshape(NP, P, D).astype(BF)
        bbbc = np.ascontiguousarray(np.tile(bb[bs][None, :], (P, 1)))
        in_maps.append(
            {
                "xT": xTc,
                "wq": wqc,
                "wk": wkc,
                "wv": wvc,
                "wb": wbc,
                "wo": woc,
                "sel": selm,
                "bbb": bbbc,
                "masks": maskm,
                "eye": eyem,
            }
        )
    return in_maps


def kernel(x, Wq, Wk, Wv, Wo, Wb, bb):
    from concourse.bass_utils import run_bass_kernel_spmd

    nc = get_nc()
    in_maps = make_core_inputs(x, Wq, Wk, Wv, Wo, Wb, bb)
    try:
        res = run_bass_kernel_spmd(nc, in_maps, core_ids=list(range(NCORES)))
    except Exception:
        # transient NRT wedges (e.g. NRT_EXEC_UNIT_UNRECOVERABLE) clear on
        # a fresh attempt; retry once before giving up
        res = run_bass_kernel_spmd(nc, in_maps, core_ids=list(range(NCORES)))
    B = 2
    y = np.zeros((B, L, D), dtype=np.float32)
    for core in range(NCORES):
        b = core // GROUPS
        y[b] += np.asarray(res.results[core]["yT"]).astype(np.float32).T
    return y


if __name__ == "__main__":
    rng = np.random.default_rng(0)
    ins = {
        "x": rng.standard_normal((2, L, D)).astype(np.float32),
        "Wq": (0.02 * rng.standard_normal((D, D))).astype(np.float32),
        "Wk": (0.02 * rng.standard_normal((D, D))).astype(np.float32),
        "Wv": (0.02 * rng.standard_normal((D, D))).astype(np.float32),
        "Wo": (0.02 * rng.standard_normal((D, D))).astype(np.float32),
        "Wb": (0.02 * rng.standard_normal((D, H))).astype(np.float32),
        "bb": np.zeros(H, dtype=np.float32),
    }
    out = kernel(**ins)
    print("kernel ran, out shape", out.shape, "mean abs", np.abs(out).mean())
